# revision 1
# baseline (speedup 1.0000x reference)
"""Trainium2 Bass kernel for nn_Correlation (81-displacement cost volume).

corr(b, d, y, x) = sum_c f1[b,c,y,x] * f2[b,c,y+dy,x+dx],  d = (dy+4)*9 + (dx+4)

Sharding: data-parallel over batch B=8, one batch per NeuronCore.

Per-core pipeline:
  1. PE band matmuls (float32r): per (y, c-chunk), lhsT = f1[c, y, :] (weights,
     M = x = 128), rhs = f2 padded rows y-4..y+4 (N = 3 dy * 136 x' per matmul),
     PSUM accumulates the two 128-channel chunks -> band M_dy[x, x'].
  2. DVE/ACT evacuate PSUM -> SBUF staging [128, 9*136].
  3. GPSIMD indirect_copy (per-16-partition-core indices): core k grabs its
     own 24-wide window [16k, 16k+24) per dy -> s1[x, (y, dy, w24)].
     (value for (x=16k+a, dxs) sits at w = a + dxs, uniform per a = x mod 16.)
  4. Ship s1 to DRAM; host does the per-lane gather w = (x mod 16) + dxs and
     the [x,(y,d)] -> [d,y,x] transpose (per-lane shear is not expressible in
     any engine AP, and this runtime has no loadable GPSIMD ucode).
"""

import sys

sys.path.insert(0, "/opt/trn_rl_repo")

from contextlib import ExitStack

import numpy as np

import jax

jax.config.update("jax_compilation_cache_dir", "/root/jaxcache")
jax.config.update("jax_persistent_cache_min_entry_size_bytes", 0)
jax.config.update("jax_persistent_cache_min_compile_time_secs", 0)

import concourse.bass as bass
import concourse.tile as tile
from concourse import bacc, mybir
from concourse.bass_utils import run_bass_kernel_spmd

F32 = mybir.dt.float32
F32R = mybir.dt.float32r
U16 = mybir.dt.uint16

C = 256
W = 128
PAD = 4
DYS = 9  # displacements per axis
WP = 144  # padded x' width (4 + 128 + 4 band + 8 align)
WIN = 32  # per-core x' window (indirect_copy inner must be pow2)
NB = 81


def build_program(H: int):
    HP = H + 2 * PAD
    nc = bacc.Bacc("TRN2", target_bir_lowering=False, debug=False)

    f1r_d = nc.dram_tensor("f1r", [H, 128, 2, W], F32R, kind="ExternalInput").ap()
    f2p_d = nc.dram_tensor("f2p", [128, 2, HP, WP], F32R, kind="ExternalInput").ap()
    idx_d = nc.dram_tensor("idx", [128, 1], U16, kind="ExternalInput").ap()
    s1_d = nc.dram_tensor("s1", [128, H * DYS * WIN], F32, kind="ExternalOutput").ap()

    with tile.TileContext(nc) as tc, ExitStack() as ctx:
        const_pool = ctx.enter_context(tc.tile_pool(name="const", bufs=1))
        f2_pool = ctx.enter_context(tc.tile_pool(name="f2", bufs=1))
        f1_pool = ctx.enter_context(tc.tile_pool(name="f1", bufs=3))
        stage_pool = ctx.enter_context(tc.tile_pool(name="stage", bufs=2))
        s1_pool = ctx.enter_context(tc.tile_pool(name="s1", bufs=1))
        psum_pool = ctx.enter_context(tc.tile_pool(name="ps", bufs=6, space="PSUM"))

        idx_t = const_pool.tile([128, 1], U16)
        nc.sync.dma_start(idx_t[:], idx_d)

        f2_t = f2_pool.tile([128, 2 * HP * WP], F32R)
        nc.sync.dma_start(f2_t[:], f2p_d)
        f2_v = f2_t[:].rearrange("p (c y x) -> p c y x", c=2, y=HP)

        s1_t = s1_pool.tile([128, H * DYS * WIN], F32)
        s1_v = s1_t[:].rearrange("p (y d w) -> p y d w", y=H, d=DYS)

        for y in range(H):
            f1_t = f1_pool.tile([128, 2 * W], F32R, tag="f1row")
            nc.sync.dma_start(f1_t[:], f1r_d[y])

            stage_t = stage_pool.tile([128, 41 * WIN], F32, tag="stg")
            for b in range(3):  # dy groups of 3
                ps = psum_pool.tile([128, 3 * WP], F32, tag="band")
                for ch in range(2):
                    nc.tensor.matmul(
                        ps[:],
                        f1_t[:, ch * W : (ch + 1) * W],
                        f2_v[:, ch, y + 3 * b : y + 3 * b + 3, :],
                        start=(ch == 0),
                        stop=(ch == 1),
                    )
                dst = stage_t[:, b * 3 * WP : (b + 1) * 3 * WP]  # 432 wide
                if b % 2 == 0:
                    nc.vector.tensor_copy(dst, ps[:])
                else:
                    nc.scalar.copy(dst, ps[:])

            nc.gpsimd.indirect_copy(
                s1_v[:, y],
                stage_t[:].rearrange("p (m w) -> p m w", w=WIN),
                idx_t[:],
                True,
            )

        nc.sync.dma_start(s1_d, s1_t[:])

    nc.compile()
    return nc


def prep_inputs(fmap1: np.ndarray, fmap2: np.ndarray):
    B, C_, H, W_ = fmap1.shape
    HP = H + 2 * PAD
    f1 = np.ascontiguousarray(fmap1, dtype=np.float32).reshape(B, 2, 128, H, W_)
    f2 = np.ascontiguousarray(fmap2, dtype=np.float32).reshape(B, 2, 128, H, W_)

    # f1r[b, y, p, ch, x] = f1[b, ch, p, y, x]
    f1r = np.transpose(f1, (0, 3, 2, 1, 4)).copy()
    # f2p[b, p, ch, yp, xp] zero-padded
    f2p = np.zeros((B, 128, 2, HP, WP), dtype=np.float32)
    f2p[:, :, :, PAD : PAD + H, PAD : PAD + W_] = np.transpose(f2, (0, 2, 1, 3, 4))

    idx = np.zeros((128, 1), dtype=np.uint16)
    for k in range(8):
        for j in range(DYS):
            idx[16 * k + j, 0] = j * WP + 16 * k
    return f1r, f2p, idx


# host-side fine shear: gathered[., x, y, dy, dxs] = s1[., x, y, dy, (x%16)+dxs]
_IDXW = (np.arange(128)[:, None] % 16 + np.arange(DYS)[None, :]).astype(np.intp)


def finish_host(s1_all: np.ndarray, H: int) -> np.ndarray:
    B = s1_all.shape[0]
    s1 = s1_all.reshape(B, 128, H, DYS, WIN)
    g = np.take_along_axis(s1, _IDXW[None, :, None, None, :], axis=4)
    # out[b, dy*9+dxs, y, x] = g[b, x, y, dy, dxs]
    return np.ascontiguousarray(g.transpose(0, 3, 4, 2, 1)).reshape(B, NB, H, s1.shape[1])


_CACHE = {}


def _get_program(H: int):
    if H not in _CACHE:
        _CACHE[H] = build_program(H)
    return _CACHE[H]


def run_on_cores(fmap1, fmap2, trace=False):
    B, C_, H, W_ = fmap1.shape
    nc = _get_program(H)
    f1r, f2p, idx = prep_inputs(fmap1, fmap2)
    in_maps = [{"f1r": f1r[b], "f2p": f2p[b], "idx": idx} for b in range(B)]
    res = run_bass_kernel_spmd(nc, in_maps, core_ids=list(range(B)), trace=trace)
    s1_all = np.stack([res.results[b]["s1"] for b in range(B)], axis=0)
    out = finish_host(s1_all, H)
    return out, res


def kernel(fmap1: np.ndarray, fmap2: np.ndarray) -> np.ndarray:
    fmap1 = np.asarray(fmap1, dtype=np.float32)
    fmap2 = np.asarray(fmap2, dtype=np.float32)
    out, _ = run_on_cores(fmap1, fmap2, trace=False)
    return out

